# revision 32
# baseline (speedup 1.0000x reference)
"""Multi-head causal attention (B=2, T=2048, C=1024, H=16) on 8 trn2 cores.

Sharding: tensor-parallel over heads. Each core computes 2 heads' QKV
projections + attention + a partial output projection; the host sums the
8 partial projections and adds the output bias.

v3 design notes (on top of the v2 p-state/filler scheme):
- att@v runs TRANSPOSED: out[q,ch] = e8.T @ va with the exp'd score block
  as the stationary operand. Output partitions become q-tokens, the full
  128x128 array is used (contraction = 128 key tokens), halving att@v PE
  cycles vs the [ch, q] orientation (out free = 65 channels only).
- The softmax row-sum falls out of the same matmuls via the ones column
  in va (out col 64), already in token-partition layout. Normalization is
  a DVE reciprocal [128,4] plus a per-head tensor_scalar_mul fused into
  the PSUM->SBUF drain. The v2 selector-matmul + Ln/Exp reciprocal
  pipeline (~20us Scalar + 8k PE cycles) is gone entirely.
- attoT2 [tok, ch] is flipped back to [ch, tok] for the output projection
  with SBUF->SBUF DMA transposes (xbar 16x128 tiles, SP queue) - no PE
  transposes, no extra PSUM, no compute-engine copies.
- QKV projection of batch 1 and the output projections are interleaved
  into the attention loops as filler so PE never idles (idle gaps cost
  ~3.4us of half-clock ramp each). QKV is emitted per (nt: x+q, k, v)
  unit for finer pump granularity; attention starts right after nt=0 of
  batch 0 so the Scalar exp stream spans nearly the whole kernel.
- Engine placement: Scalar = exp only (+ a few DMA issues); DVE = bias
  copies, masks, recips, half the outproj drains; Pool = scaled attoT2
  copies + other half of outproj drains; SP = bulk DMA + transposes.
"""

import contextlib
import os

import ml_dtypes
import numpy as np

import bass_rust
import concourse.bass as bass
import concourse.mybir as mybir
import concourse.tile as tile
from concourse.bass_utils import run_bass_kernel_spmd

F32 = mybir.dt.float32
BF16 = mybir.dt.bfloat16
NPBF16 = ml_dtypes.bfloat16

B, T, C, H = 2, 2048, 1024, 16
D = C // H          # 64
NCORES = 8
HL = H // NCORES    # heads per core = 2
TOK = B * T         # 4096
HC = HL * D         # local head channels = 128

NT = T // 512       # 4 token column tiles (512) per batch
KT = C // 128       # 8 contraction tiles for projections
QT = T // 512       # 4 q tiles per batch
JBB = T // 128      # 16 j (key) blocks per batch

_MAXW = 1


def _patched_drain_and_barrier(self, tick_clock, wait_clock):
    """Stock tile tail drain carries one sem-wait per outstanding proc on a
    single TPB_CTRL drain; this walrus build allows only one sync-wait per
    ctrl instruction. Split the waits across no-op carriers."""
    nc = self.nc
    carrier = nc.sync.nop()
    wait_clock.add_sem_waits(
        carrier.ins, bass_rust.ScopedClock({None: tick_clock.global_clock})
    )
    si = carrier.ins.sync_info
    waits = list(si.on_wait) if si and si.on_wait else []
    if len(waits) > _MAXW:
        carrier.ins.sync_info = mybir.SyncInfo(
            on_wait=waits[:_MAXW], on_update=list(si.on_update or [])
        )
        for i in range(_MAXW, len(waits), _MAXW):
            nop = nc.sync.nop()
            nop.ins.sync_info = mybir.SyncInfo(
                on_wait=waits[i : i + _MAXW], on_update=[]
            )
    nc.sync.drain()

    nc.all_engine_barrier()
    popped = nc._tile_sem_poison_stack.pop()
    assert popped is self._sem_poison
    assert self.sems is not None
    nc.clear_and_free_semaphores(list(self.sems.allocated().values()))
    nc.all_engine_barrier()


tile.TileContext._drain_and_barrier = _patched_drain_and_barrier


def _split_waits(nc, maxw=_MAXW):
    """This walrus build accepts at most one sync-wait per instruction.
    Hoist excess waits onto no-op carriers inserted just before the
    instruction on the same engine."""
    for f in nc.m.functions:
        for bb in f.blocks:
            insts = bb.instructions
            if not any(
                i.sync_info and i.sync_info.on_wait and len(i.sync_info.on_wait) > maxw
                for i in insts
            ):
                continue
            new = []
            for inst in insts:
                si = inst.sync_info
                waits = list(si.on_wait) if si and si.on_wait else []
                if len(waits) > maxw:
                    keep = waits[-maxw:]
                    extra = waits[:-maxw]
                    for j in range(0, len(extra), maxw):
                        nop = mybir.InstNoOp(name=nc.get_next_instruction_name())
                        nop.engine = inst.engine
                        nop.sync_info = mybir.SyncInfo(
                            on_wait=extra[j : j + maxw], on_update=[]
                        )
                        nc.register_instruction(nop)
                        new.append(nop)
                    inst.sync_info = mybir.SyncInfo(
                        on_wait=keep, on_update=list(si.on_update or [])
                    )
                new.append(inst)
            bb.instructions = new


def build():
    nc = bass.Bass()
    xT = nc.declare_dram_parameter("xT", [C, TOK], BF16, isOutput=False)
    wq = nc.declare_dram_parameter("wq", [C, HC], BF16, isOutput=False)
    wk = nc.declare_dram_parameter("wk", [C, HC], BF16, isOutput=False)
    wv = nc.declare_dram_parameter("wv", [C, HC], BF16, isOutput=False)
    wo = nc.declare_dram_parameter("wo", [HC, C], BF16, isOutput=False)
    bq = nc.declare_dram_parameter("bq", [HC, 1], F32, isOutput=False)
    bk = nc.declare_dram_parameter("bk", [HC, 1], F32, isOutput=False)
    bv = nc.declare_dram_parameter("bv", [1, HC], BF16, isOutput=False)
    ones1 = nc.declare_dram_parameter("ones1", [1, 128], BF16, isOutput=False)
    masks = nc.declare_dram_parameter("masks", [4, 128, 512], BF16, isOutput=False)
    out = nc.declare_dram_parameter("out", [TOK, C], BF16, isOutput=True)

    Exp = mybir.ActivationFunctionType.Exp

    with contextlib.ExitStack() as _st:
        _st.enter_context(
            nc.allow_low_precision(reason="bf16 matmuls with fp32 accumulation")
        )
        tc = _st.enter_context(tile.TileContext(nc))
        with (
            tc.tile_pool(name="consts", bufs=1) as consts,
            tc.tile_pool(name="persist", bufs=1) as persist,
            tc.tile_pool(name="work", bufs=2) as work,
            tc.tile_pool(name="ps_proj", bufs=2, space="PSUM") as ps_proj,
            tc.tile_pool(name="ps_s", bufs=2, space="PSUM") as ps_s,
            tc.tile_pool(name="ps_o", bufs=2, space="PSUM") as ps_o,
        ):
            # ---- constants into SBUF ----
            wq_sb = consts.tile([128, KT, 128], BF16, name="wq_sb")
            wk_sb = consts.tile([128, KT, 128], BF16, name="wk_sb")
            wv_sb = consts.tile([128, KT, 128], BF16, name="wv_sb")
            # first chunks of each weight lead so the first projection
            # matmuls start early; small consts + masks on the scalar queue
            # (idle until the first exp), wo (needed last) at the back
            wtriples = [
                (wq_sb, wq.rearrange("(a p) m -> p a m", p=128)),
                (wk_sb, wk.rearrange("(a p) m -> p a m", p=128)),
                (wv_sb, wv.rearrange("(a p) m -> p a m", p=128)),
            ]
            for w_sb, wr in wtriples:
                nc.sync.dma_start(w_sb[:, 0:2], wr[:, 0:2])
            for w_sb, wr in wtriples:
                nc.sync.dma_start(w_sb[:, 2:5], wr[:, 2:5])
                nc.sync.dma_start(w_sb[:, 5:KT], wr[:, 5:KT])
            wo_sb = consts.tile([128, C], BF16, name="wo_sb")
            bq_sb = consts.tile([HC, 1], F32, name="bq_sb")
            bk_sb = consts.tile([HC, 1], F32, name="bk_sb")
            for b_sb, b_dr in ((bq_sb, bq), (bk_sb, bk)):
                nc.scalar.dma_start(b_sb, b_dr[:])
            bv_sb = consts.tile([1, HC], BF16, name="bv_sb")
            nc.scalar.dma_start(bv_sb, bv[:])
            ones1_sb = consts.tile([1, 128], BF16, name="ones1_sb")
            nc.scalar.dma_start(ones1_sb, ones1[:])
            masks_sb = consts.tile([128, 4, 512], BF16, name="masks_sb")

            # ---- persistent activations ----
            # per-batch / per-qtile tiles: one big tile would funnel all
            # readers+writers through the dep tracker's conservative
            # whole-tile fallback (WAR on late batch-0 outproj reads was
            # stalling batch-1 transposes ~19us, head-of-line blocking the
            # issuing engine queue behind them)
            qTs = [persist.tile([HC, T], BF16, name=f"qT{b}") for b in range(B)]
            kTs = [persist.tile([HC, T], BF16, name=f"kT{b}") for b in range(B)]
            attoTs = {
                (b, i): persist.tile([HC, 4, 128], BF16, name=f"attoT{b}_{i}")
                for b in range(B)
                for i in range(QT)
            }
            # va: per local head, [key-token partitions, 32 global blocks,
            # 64 v channels + ones column], pitch 80
            va_sb = persist.tile([128, 2 * JBB, HL, 80], BF16, name="va_sb")
            ones_scr = consts.tile([128, 2 * JBB, HL], BF16, name="ones_scr")
            nc.vector.memset(ones_scr, 1.0)
            nc.vector.tensor_copy(va_sb[:, :, :, D], ones_scr)

            # ================= emission units =================

            xTr = xT.rearrange("(a p) m -> p a m", p=128)
            xpend = {}

            def xload(b, nt):
                """Prefetch one 512-token x tile (issued one proj unit ahead
                so the q matmuls never wait on HBM)."""
                c0 = b * T + nt * 512
                xrow = work.tile(
                    [128, KT, 512], BF16, tag="xcol", bufs=3, name="xrow"
                )
                nc.gpsimd.dma_start(xrow[:, 0:2], xTr[:, 0:2, c0 : c0 + 512])
                nc.gpsimd.dma_start(xrow[:, 2:KT], xTr[:, 2:KT, c0 : c0 + 512])
                xpend[(b, nt)] = xrow

            def unit_q(b, nt, xrow, half):
                """Half a Q projection (4 contraction chunks)."""
                if half == 0:
                    q_ps = ps_proj.tile([128, 512], F32, tag="proj", name="q_ps")
                    state["q_ps"] = q_ps
                else:
                    q_ps = state["q_ps"]
                for kt in range(4 * half, 4 * half + 4):
                    nc.tensor.matmul(
                        q_ps, lhsT=wq_sb[:, kt, :], rhs=xrow[:, kt, :],
                        start=(kt == 0), stop=(kt == KT - 1),
                    )
                if half == 1:
                    nc.vector.tensor_scalar_add(
                        qTs[b][:, nt * 512 : nt * 512 + 512], q_ps, bq_sb
                    )

            def unit_k(b, nt, xrow, half):
                if half == 0:
                    k_ps = ps_proj.tile([128, 512], F32, tag="proj", name="k_ps")
                    state["k_ps"] = k_ps
                else:
                    k_ps = state["k_ps"]
                for kt in range(4 * half, 4 * half + 4):
                    nc.tensor.matmul(
                        k_ps, lhsT=wk_sb[:, kt, :], rhs=xrow[:, kt, :],
                        start=(kt == 0), stop=(kt == KT - 1),
                    )
                if half == 1:
                    nc.vector.tensor_scalar_add(
                        kTs[b][:, nt * 512 : nt * 512 + 512], k_ps, bk_sb
                    )

            def unit_v(b, nt, xrow, blk):
                """v in [token, channel] layout: x block as stationary."""
                gblk = b * JBB + nt * 4 + blk
                va_ps = ps_proj.tile([128, 128], F32, tag="proj", name="va_ps")
                nc.tensor.matmul(
                    va_ps, lhsT=ones1_sb, rhs=bv_sb, start=True, stop=False
                )
                for kt in range(KT):
                    nc.tensor.matmul(
                        va_ps,
                        lhsT=xrow[:, kt, blk * 128 : (blk + 1) * 128],
                        rhs=wv_sb[:, kt, :],
                        start=False,
                        stop=(kt == KT - 1),
                    )
                nc.vector.tensor_copy(
                    va_sb[:, gblk, :, 0:D],
                    va_ps.rearrange("p (h d) -> p h d", h=HL),
                )

            def proj_units():
                for b in range(B):
                    for nt in range(NT):
                        holder = {}

                        def u0(b=b, nt=nt, holder=holder):
                            holder["x"] = xpend.pop((b, nt))
                            nxt = (b, nt + 1) if nt + 1 < NT else (b + 1, 0)
                            if nxt[0] < B:
                                xload(*nxt)
                            unit_q(b, nt, holder["x"], 0)

                        yield u0
                        yield lambda b=b, nt=nt, holder=holder: unit_q(b, nt, holder["x"], 1)
                        yield lambda b=b, nt=nt, holder=holder: unit_k(b, nt, holder["x"], 0)
                        yield lambda b=b, nt=nt, holder=holder: unit_k(b, nt, holder["x"], 1)
                        for blk in range(4):
                            yield lambda b=b, nt=nt, blk=blk, holder=holder: unit_v(
                                b, nt, holder["x"], blk
                            )

            def attn_pairs(b, hl, i, pump, deferred):
                """Scores+exp for one (batch, local head, 512-query tile).
                Diagonal pairs first. The previous group's deferred att@v
                bursts are emitted after pair 1's scores so the exps they
                depend on are comfortably drained. Returns e8 tiles by pair.

                PSUM accumulation groups may not interleave their open spans
                within one bank (verified on hw: interleaved start/stop at
                different offsets of one bank corrupts all groups that don't
                close on the bank's final matmul). att@v therefore runs
                s-major as contiguous bursts - one open group at a time -
                pipelined one group behind the scores/exp stream."""
                t0 = b * T
                h0 = hl * D
                q0 = t0 + i * 512
                order = [2 * i, 2 * i + 1] + list(range(0, 2 * i))
                npair = len(order)
                # both diagonal pairs share one e8 tile so a single DVE mul
                # masks all four triangle blocks (the [1024:1280] gap region
                # is exp'd garbage that no burst ever reads)
                e8d = work.tile([128, 4, 512], BF16, tag="esd", bufs=4, name="e8d")
                edflat = e8d.rearrange("p a f -> p (a f)")
                e8s = {}
                for idx, p in enumerate(order):
                    jbs = (2 * p, 2 * p + 1)
                    css = [max(0, 128 * (jb - 4 * i)) for jb in jbs]
                    cs = css[0]
                    diag = jbs[0] >= 4 * i
                    s_ps = ps_s.tile([128, 2, 512], F32, tag="sps", name="s_ps")
                    q0l = i * 512
                    for j, jb in enumerate(jbs):
                        nc.tensor.matmul(
                            s_ps[:, j, css[j] : 512],
                            lhsT=kTs[b][
                                h0 : h0 + D, jb * 128 : (jb + 1) * 128
                            ],
                            rhs=qTs[b][h0 : h0 + D, q0l + css[j] : q0l + 512],
                            start=True,
                            stop=True,
                        )
                    sflat = s_ps.rearrange("p a f -> p (a f)")
                    if diag:
                        pp = (jbs[0] - 4 * i) // 2
                        e8s[p] = (e8d, 2 * pp)
                        nc.scalar.activation(
                            edflat[:, 1024 * pp + cs : 1024 * pp + 1024],
                            sflat[:, cs:1024],
                            Exp,
                            scale=0.125,
                        )
                        if pp == 1:
                            mflat = masks_sb.rearrange("p r f -> p (r f)")
                            nc.vector.tensor_mul(edflat, edflat, mflat)
                    else:
                        e8 = work.tile(
                            [128, 2, 512], BF16, tag="esb", bufs=12, name="e8"
                        )
                        e8s[p] = (e8, 0)
                        eflat = e8.rearrange("p a f -> p (a f)")
                        nc.scalar.activation(
                            eflat, sflat, Exp, scale=0.125
                        )
                    if idx >= 1 and deferred:
                        if idx < npair - 1:
                            deferred.pop(0)()
                            if deferred:
                                deferred.pop(0)()
                        else:
                            while deferred:
                                deferred.pop(0)()
                    pump()
                return e8s

            def attv2_burst(b, hl, i, o2, e8s, s):
                """One qsub's att@v accumulation: a single contiguous
                open-close psum group over key blocks 0..4i+s."""
                for kb in range(4 * i + s + 1):
                    p, j = divmod(kb, 2)
                    tile_, j0 = e8s[p]
                    nc.tensor.matmul(
                        o2[:, s, 0:65],
                        lhsT=tile_[:, j0 + j, 128 * s : 128 * (s + 1)],
                        rhs=va_sb[:, b * JBB + kb, hl, 0 : D + 1],
                        start=(kb == 0),
                        stop=(kb == 4 * i + s),
                        skip_group_check=True,
                    )

            def post_hl(b, hl, i, o2, att2, r_sb):
                """Normalize+drain one head's attv2 psum into attoT2; on the
                second head, chase each qsub's copy with its DMA-transpose
                into attoT [ch, tok] (two queues so the four transposes run
                pairwise-parallel)."""
                nc.vector.reciprocal(r_sb[:, :], o2[:, 0:4, 64])
                for s in range(4):
                    nc.vector.tensor_scalar_mul(
                        att2[:, s, h0c(hl)], o2[:, s, 0:64], r_sb[:, s : s + 1]
                    )
                if hl == 1:
                    # one batched xbar transpose flips all four [tok, ch]
                    # blocks into attoT's [ch, tok] layout
                    nc.sync.dma_start_transpose(
                        attoTs[(b, i)], att2.rearrange("p a f -> p (a f)")
                    )

            def h0c(hl):
                return slice(hl * D, (hl + 1) * D)

            def outproj_tile(b, tt, k):
                """One [128 tok, 1024 C] partial output projection block."""
                t0 = b * T
                tb = t0 // 128 + tt
                o_sb = work.tile(
                    [128, 2, 512], BF16, tag="osb", bufs=3, name="o_sb"
                )
                for no2 in range(2):
                    p_ps = ps_proj.tile([128, 512], F32, tag="proj", name="p_ps")
                    nc.tensor.matmul(
                        p_ps,
                        lhsT=attoTs[(b, tt // 4)][:, tt % 4, :],
                        rhs=wo_sb[:, no2 * 512 : (no2 + 1) * 512],
                        start=True,
                        stop=True,
                    )
                    # gpsimd can't read PSUM; scalar jitter would stall the
                    # scores ring through the exp chain: drains live on DVE
                    nc.vector.tensor_copy(o_sb[:, no2, :], p_ps)
                nc.gpsimd.dma_start(
                    out[tb * 128 : (tb + 1) * 128, :],
                    o_sb.rearrange("p a f -> p (a f)"),
                )

            # ================= schedule =================
            # two filler queues: outproj units (dependency-lagged, preferred)
            # and proj units (dependency-free but deadline-bound). Proj units
            # are only pumped when their deadline is near, reserving them as
            # dense filler for the outproj-poor attention groups they unlock.
            filler = []
            state = {"proj_done": 0, "opk": 0, "cur": 0}
            units = proj_units()

            def run_proj_unit():
                next(units)()
                state["proj_done"] += 1

            def pump():
                if filler:
                    filler.pop(0)()
                    return
                # proj unit n serves attn group n//3 (3 units per nt)
                if state["proj_done"] < 8 * B * NT and (
                    state["proj_done"] // 8 <= state["cur"] + 2
                ):
                    run_proj_unit()

            def force_proj(b, i):
                need = 8 * (i + 1) + (8 * NT if b == 1 else 0)
                while state["proj_done"] < need:
                    run_proj_unit()

            # startup: first projection tile of batch 0, no filler
            _s = nc.enter_named_scope("phaseA0", True)
            xload(0, 0)
            for _ in range(8):
                run_proj_unit()
            nc.scalar.dma_start(masks_sb, masks.rearrange("r p f -> p r f"))
            nc.scalar.dma_start(wo_sb[:, 0:512], wo[:, 0:512])
            nc.scalar.dma_start(wo_sb[:, 512:C], wo[:, 512:C])
            nc.leave_named_scope("phaseA0", _s[0], True)

            deferred = []
            staging = []
            for b in range(B):
                _s = nc.enter_named_scope(f"attn{b}", True)
                for i in range(QT):
                    state["cur"] = b * QT + i
                    force_proj(b, i)
                    if b == 0 and i < 2:
                        for _ in range(4 - 2 * i):
                            pump()
                    att2 = work.tile(
                        [128, 4, 128], BF16, tag="att2", bufs=2, name="att2"
                    )
                    for hl in range(HL):
                        filler.extend(staging)
                        staging.clear()
                        r_sb = work.tile(
                            [128, 4], F32, tag="rsb", bufs=4, name="r_sb"
                        )
                        o2 = ps_o.tile([128, 4, 128], F32, tag="ops", name="o2")
                        e8s = attn_pairs(b, hl, i, pump, deferred)
                        # defer this group's att@v + normalization into the
                        # next group's pair loop (past its exp latency)
                        deferred = [
                            (lambda b=b, hl=hl, i=i, o2=o2, e8s=e8s, s=s:
                                attv2_burst(b, hl, i, o2, e8s, s))
                            for s in range(4)
                        ]
                        deferred.append(
                            lambda b=b, hl=hl, i=i, o2=o2, att2=att2, r_sb=r_sb:
                                post_hl(b, hl, i, o2, att2, r_sb)
                        )
                        if hl == 1:
                            def fin(b=b, i=i):
                                # stage rather than release: outproj units
                                # become poppable one group later, past the
                                # copy->transpose chain of their attoT data
                                for tt in range(i * 4, (i + 1) * 4):
                                    k = state["opk"]
                                    state["opk"] += 1
                                    staging.append(
                                        lambda b=b, tt=tt, k=k:
                                            outproj_tile(b, tt, k)
                                    )
                            deferred.append(fin)
                nc.leave_named_scope(f"attn{b}", _s[0], True)

            _s = nc.enter_named_scope("tail", True)
            # backlog first: it covers the last group's exp/copy latency
            while state["proj_done"] < 8 * B * NT:
                run_proj_unit()
            while filler:
                filler.pop(0)()
            while deferred:
                deferred.pop(0)()
            filler.extend(staging)
            staging.clear()
            while filler:
                filler.pop(0)()
            nc.leave_named_scope("tail", _s[0], True)

    _split_waits(nc)
    return nc


def make_in_maps(x, Wq, bq, Wk, bk, Wv, bv, Wo, bo):
    xT = np.ascontiguousarray(x.reshape(TOK, C).T).astype(NPBF16)
    # masks[r, a, c] = 1 if c >= 128r + a  (causal within diagonal blocks)
    a = np.arange(128)[:, None]
    c = np.arange(512)[None, :]
    masks = np.stack(
        [(c >= 128 * rr + a).astype(NPBF16) for rr in range(4)]
    )
    in_maps = []
    for core in range(NCORES):
        sl = slice(core * HC, (core + 1) * HC)
        in_maps.append(
            {
                "xT": xT,
                "wq": np.ascontiguousarray(Wq[sl, :].T).astype(NPBF16),
                "wk": np.ascontiguousarray(Wk[sl, :].T).astype(NPBF16),
                "wv": np.ascontiguousarray(Wv[sl, :].T).astype(NPBF16),
                "wo": np.ascontiguousarray(Wo[:, sl].T).astype(NPBF16),
                "bq": np.ascontiguousarray(bq[sl]).reshape(HC, 1),
                "bk": np.ascontiguousarray(bk[sl]).reshape(HC, 1),
                "bv": np.ascontiguousarray(bv[sl]).reshape(1, HC).astype(NPBF16),
                "ones1": np.ones((1, 128), NPBF16),
                "masks": masks,
            }
        )
    return in_maps


_NC_CACHE = None


def kernel(x, Wq, bq, Wk, bk, Wv, bv, Wo, bo):
    global _NC_CACHE
    x = np.asarray(x, np.float32)
    in_maps = make_in_maps(
        x,
        np.asarray(Wq, np.float32),
        np.asarray(bq, np.float32),
        np.asarray(Wk, np.float32),
        np.asarray(bk, np.float32),
        np.asarray(Wv, np.float32),
        np.asarray(bv, np.float32),
        np.asarray(Wo, np.float32),
        np.asarray(bo, np.float32),
    )
    if _NC_CACHE is None:
        _NC_CACHE = build()
    trace = bool(int(os.environ.get("KERNEL_TRACE", "0")))
    res = run_bass_kernel_spmd(
        _NC_CACHE, in_maps, core_ids=list(range(NCORES)), trace=trace
    )
    if trace:
        kernel.last_results = res
    total = np.zeros((TOK, C), np.float32)
    for core in range(NCORES):
        total += res.results[core]["out"].astype(np.float32)
    total += np.asarray(bo, np.float32)[None, :]
    return total.reshape(B, T, C)


# revision 33
# speedup vs baseline: 1.0283x; 1.0283x over previous
"""Multi-head causal attention (B=2, T=2048, C=1024, H=16) on 8 trn2 cores.

Sharding: tensor-parallel over heads. Each core computes 2 heads' QKV
projections + attention + a partial output projection; the host sums the
8 partial projections and adds the output bias.

v3 design notes (on top of the v2 p-state/filler scheme):
- att@v runs TRANSPOSED: out[q,ch] = e8.T @ va with the exp'd score block
  as the stationary operand. Output partitions become q-tokens, the full
  128x128 array is used (contraction = 128 key tokens), halving att@v PE
  cycles vs the [ch, q] orientation (out free = 65 channels only).
- The softmax row-sum falls out of the same matmuls via the ones column
  in va (out col 64), already in token-partition layout. Normalization is
  a DVE reciprocal [128,4] plus a per-head tensor_scalar_mul fused into
  the PSUM->SBUF drain. The v2 selector-matmul + Ln/Exp reciprocal
  pipeline (~20us Scalar + 8k PE cycles) is gone entirely.
- attoT2 [tok, ch] is flipped back to [ch, tok] for the output projection
  with SBUF->SBUF DMA transposes (xbar 16x128 tiles, SP queue) - no PE
  transposes, no extra PSUM, no compute-engine copies.
- QKV projection of batch 1 and the output projections are interleaved
  into the attention loops as filler so PE never idles (idle gaps cost
  ~3.4us of half-clock ramp each). QKV is emitted per (nt: x+q, k, v)
  unit for finer pump granularity; attention starts right after nt=0 of
  batch 0 so the Scalar exp stream spans nearly the whole kernel.
- Engine placement: Scalar = exp only (+ a few DMA issues); DVE = bias
  copies, masks, recips, half the outproj drains; Pool = scaled attoT2
  copies + other half of outproj drains; SP = bulk DMA + transposes.
"""

import contextlib
import os

import ml_dtypes
import numpy as np

import bass_rust
import concourse.bass as bass
import concourse.mybir as mybir
import concourse.tile as tile
from concourse.bass_utils import run_bass_kernel_spmd

F32 = mybir.dt.float32
BF16 = mybir.dt.bfloat16
NPBF16 = ml_dtypes.bfloat16

B, T, C, H = 2, 2048, 1024, 16
D = C // H          # 64
NCORES = 8
HL = H // NCORES    # heads per core = 2
TOK = B * T         # 4096
HC = HL * D         # local head channels = 128

NT = T // 512       # 4 token column tiles (512) per batch
KT = C // 128       # 8 contraction tiles for projections
QT = T // 512       # 4 q tiles per batch
JBB = T // 128      # 16 j (key) blocks per batch

_MAXW = 1


def _patched_drain_and_barrier(self, tick_clock, wait_clock):
    """Stock tile tail drain carries one sem-wait per outstanding proc on a
    single TPB_CTRL drain; this walrus build allows only one sync-wait per
    ctrl instruction. Split the waits across no-op carriers."""
    nc = self.nc
    carrier = nc.sync.nop()
    wait_clock.add_sem_waits(
        carrier.ins, bass_rust.ScopedClock({None: tick_clock.global_clock})
    )
    si = carrier.ins.sync_info
    waits = list(si.on_wait) if si and si.on_wait else []
    if len(waits) > _MAXW:
        carrier.ins.sync_info = mybir.SyncInfo(
            on_wait=waits[:_MAXW], on_update=list(si.on_update or [])
        )
        for i in range(_MAXW, len(waits), _MAXW):
            nop = nc.sync.nop()
            nop.ins.sync_info = mybir.SyncInfo(
                on_wait=waits[i : i + _MAXW], on_update=[]
            )
    nc.sync.drain()

    nc.all_engine_barrier()
    popped = nc._tile_sem_poison_stack.pop()
    assert popped is self._sem_poison
    assert self.sems is not None
    nc.clear_and_free_semaphores(list(self.sems.allocated().values()))
    nc.all_engine_barrier()


tile.TileContext._drain_and_barrier = _patched_drain_and_barrier


def _split_waits(nc, maxw=_MAXW):
    """This walrus build accepts at most one sync-wait per instruction.
    Hoist excess waits onto no-op carriers inserted just before the
    instruction on the same engine."""
    for f in nc.m.functions:
        for bb in f.blocks:
            insts = bb.instructions
            if not any(
                i.sync_info and i.sync_info.on_wait and len(i.sync_info.on_wait) > maxw
                for i in insts
            ):
                continue
            new = []
            for inst in insts:
                si = inst.sync_info
                waits = list(si.on_wait) if si and si.on_wait else []
                if len(waits) > maxw:
                    keep = waits[-maxw:]
                    extra = waits[:-maxw]
                    for j in range(0, len(extra), maxw):
                        nop = mybir.InstNoOp(name=nc.get_next_instruction_name())
                        nop.engine = inst.engine
                        nop.sync_info = mybir.SyncInfo(
                            on_wait=extra[j : j + maxw], on_update=[]
                        )
                        nc.register_instruction(nop)
                        new.append(nop)
                    inst.sync_info = mybir.SyncInfo(
                        on_wait=keep, on_update=list(si.on_update or [])
                    )
                new.append(inst)
            bb.instructions = new


def build():
    nc = bass.Bass()
    xT = nc.declare_dram_parameter("xT", [C, TOK], BF16, isOutput=False)
    wq = nc.declare_dram_parameter("wq", [C, HC], BF16, isOutput=False)
    wk = nc.declare_dram_parameter("wk", [C, HC], BF16, isOutput=False)
    wv = nc.declare_dram_parameter("wv", [C, HC], BF16, isOutput=False)
    wo = nc.declare_dram_parameter("wo", [HC, C], BF16, isOutput=False)
    bq = nc.declare_dram_parameter("bq", [HC, 1], F32, isOutput=False)
    bk = nc.declare_dram_parameter("bk", [HC, 1], F32, isOutput=False)
    bv = nc.declare_dram_parameter("bv", [1, HC], BF16, isOutput=False)
    ones1 = nc.declare_dram_parameter("ones1", [1, 128], BF16, isOutput=False)
    masks = nc.declare_dram_parameter("masks", [4, 128, 512], BF16, isOutput=False)
    out = nc.declare_dram_parameter("out", [TOK, C], BF16, isOutput=True)

    Exp = mybir.ActivationFunctionType.Exp

    with contextlib.ExitStack() as _st:
        _st.enter_context(
            nc.allow_low_precision(reason="bf16 matmuls with fp32 accumulation")
        )
        tc = _st.enter_context(tile.TileContext(nc))
        with (
            tc.tile_pool(name="consts", bufs=1) as consts,
            tc.tile_pool(name="persist", bufs=1) as persist,
            tc.tile_pool(name="work", bufs=2) as work,
            tc.tile_pool(name="ps_proj", bufs=2, space="PSUM") as ps_proj,
            tc.tile_pool(name="ps_s", bufs=2, space="PSUM") as ps_s,
            tc.tile_pool(name="ps_o", bufs=2, space="PSUM") as ps_o,
        ):
            # ---- constants into SBUF ----
            wq_sb = consts.tile([128, KT, 128], BF16, name="wq_sb")
            wk_sb = consts.tile([128, KT, 128], BF16, name="wk_sb")
            wv_sb = consts.tile([128, KT, 128], BF16, name="wv_sb")
            # first chunks of each weight lead so the first projection
            # matmuls start early; small consts + masks on the scalar queue
            # (idle until the first exp), wo (needed last) at the back
            wtriples = [
                (wq_sb, wq.rearrange("(a p) m -> p a m", p=128)),
                (wk_sb, wk.rearrange("(a p) m -> p a m", p=128)),
                (wv_sb, wv.rearrange("(a p) m -> p a m", p=128)),
            ]
            for w_sb, wr in wtriples:
                nc.sync.dma_start(w_sb[:, 0:2], wr[:, 0:2])
            for w_sb, wr in wtriples:
                nc.sync.dma_start(w_sb[:, 2:5], wr[:, 2:5])
                nc.sync.dma_start(w_sb[:, 5:KT], wr[:, 5:KT])
            wo_sb = consts.tile([128, C], BF16, name="wo_sb")
            bq_sb = consts.tile([HC, 1], F32, name="bq_sb")
            bk_sb = consts.tile([HC, 1], F32, name="bk_sb")
            for b_sb, b_dr in ((bq_sb, bq), (bk_sb, bk)):
                nc.scalar.dma_start(b_sb, b_dr[:])
            bv_sb = consts.tile([1, HC], BF16, name="bv_sb")
            nc.scalar.dma_start(bv_sb, bv[:])
            ones1_sb = consts.tile([1, 128], BF16, name="ones1_sb")
            nc.scalar.dma_start(ones1_sb, ones1[:])
            masks_sb = consts.tile([128, 4, 512], BF16, name="masks_sb")

            # ---- persistent activations ----
            # per-batch / per-qtile tiles: one big tile would funnel all
            # readers+writers through the dep tracker's conservative
            # whole-tile fallback (WAR on late batch-0 outproj reads was
            # stalling batch-1 transposes ~19us, head-of-line blocking the
            # issuing engine queue behind them)
            qTs = [persist.tile([HC, T], BF16, name=f"qT{b}") for b in range(B)]
            kTs = [persist.tile([HC, T], BF16, name=f"kT{b}") for b in range(B)]
            attoTs = {
                (b, i): persist.tile([HC, 4, 128], BF16, name=f"attoT{b}_{i}")
                for b in range(B)
                for i in range(QT)
            }
            # va: per local head, [key-token partitions, 32 global blocks,
            # 64 v channels + ones column], pitch 80
            va_sb = persist.tile([128, 2 * JBB, HL, 80], BF16, name="va_sb")
            ones_scr = consts.tile([128, 2 * JBB, HL], BF16, name="ones_scr")
            nc.vector.memset(ones_scr, 1.0)
            nc.vector.tensor_copy(va_sb[:, :, :, D], ones_scr)

            # ================= emission units =================

            xTr = xT.rearrange("(a p) m -> p a m", p=128)
            xpend = {}

            def xload(b, nt):
                """Prefetch one 512-token x tile (issued one proj unit ahead
                so the q matmuls never wait on HBM)."""
                c0 = b * T + nt * 512
                xrow = work.tile(
                    [128, KT, 512], BF16, tag="xcol", bufs=3, name="xrow"
                )
                nc.gpsimd.dma_start(xrow[:, 0:2], xTr[:, 0:2, c0 : c0 + 512])
                nc.gpsimd.dma_start(xrow[:, 2:KT], xTr[:, 2:KT, c0 : c0 + 512])
                xpend[(b, nt)] = xrow

            def unit_xq(b, nt):
                """Q projection for one 512-token tile (x already in SBUF)."""
                xrow = xpend.pop((b, nt))
                nxt = (b, nt + 1) if nt + 1 < NT else (b + 1, 0)
                if nxt[0] < B:
                    xload(*nxt)
                q_ps = ps_proj.tile([128, 512], F32, tag="proj", name="q_ps")
                for kt in range(KT):
                    nc.tensor.matmul(
                        q_ps, lhsT=wq_sb[:, kt, :], rhs=xrow[:, kt, :],
                        start=(kt == 0), stop=(kt == KT - 1),
                    )
                nc.vector.tensor_scalar_add(
                    qTs[b][:, nt * 512 : nt * 512 + 512], q_ps, bq_sb
                )
                return xrow

            def unit_k(b, nt, xrow):
                k_ps = ps_proj.tile([128, 512], F32, tag="proj", name="k_ps")
                for kt in range(KT):
                    nc.tensor.matmul(
                        k_ps, lhsT=wk_sb[:, kt, :], rhs=xrow[:, kt, :],
                        start=(kt == 0), stop=(kt == KT - 1),
                    )
                nc.vector.tensor_scalar_add(
                    kTs[b][:, nt * 512 : nt * 512 + 512], k_ps, bk_sb
                )

            def unit_v(b, nt, xrow):
                """v in [token, channel] layout: x block as stationary."""
                for blk in range(4):
                    gblk = b * JBB + nt * 4 + blk
                    va_ps = ps_proj.tile([128, 128], F32, tag="proj", name="va_ps")
                    nc.tensor.matmul(
                        va_ps, lhsT=ones1_sb, rhs=bv_sb, start=True, stop=False
                    )
                    for kt in range(KT):
                        nc.tensor.matmul(
                            va_ps,
                            lhsT=xrow[:, kt, blk * 128 : (blk + 1) * 128],
                            rhs=wv_sb[:, kt, :],
                            start=False,
                            stop=(kt == KT - 1),
                        )
                    nc.vector.tensor_copy(
                        va_sb[:, gblk, :, 0:D],
                        va_ps.rearrange("p (h d) -> p h d", h=HL),
                    )

            def proj_units():
                for b in range(B):
                    for nt in range(NT):
                        holder = {}

                        def uxq(b=b, nt=nt, holder=holder):
                            holder["x"] = unit_xq(b, nt)

                        yield uxq
                        yield lambda b=b, nt=nt, holder=holder: unit_k(b, nt, holder["x"])
                        yield lambda b=b, nt=nt, holder=holder: unit_v(b, nt, holder["x"])

            def attn_pairs(b, hl, i, pump, deferred):
                """Scores+exp for one (batch, local head, 512-query tile).
                Diagonal pairs first. The previous group's deferred att@v
                bursts are emitted after pair 1's scores so the exps they
                depend on are comfortably drained. Returns e8 tiles by pair.

                PSUM accumulation groups may not interleave their open spans
                within one bank (verified on hw: interleaved start/stop at
                different offsets of one bank corrupts all groups that don't
                close on the bank's final matmul). att@v therefore runs
                s-major as contiguous bursts - one open group at a time -
                pipelined one group behind the scores/exp stream."""
                t0 = b * T
                h0 = hl * D
                q0 = t0 + i * 512
                order = [2 * i, 2 * i + 1] + list(range(0, 2 * i))
                npair = len(order)
                # both diagonal pairs share one e8 tile so a single DVE mul
                # masks all four triangle blocks (the [1024:1280] gap region
                # is exp'd garbage that no burst ever reads)
                e8d = work.tile([128, 4, 512], BF16, tag="esd", bufs=4, name="e8d")
                edflat = e8d.rearrange("p a f -> p (a f)")
                e8s = {}
                for idx, p in enumerate(order):
                    jbs = (2 * p, 2 * p + 1)
                    css = [max(0, 128 * (jb - 4 * i)) for jb in jbs]
                    cs = css[0]
                    diag = jbs[0] >= 4 * i
                    s_ps = ps_s.tile([128, 2, 512], F32, tag="sps", name="s_ps")
                    q0l = i * 512
                    for j, jb in enumerate(jbs):
                        nc.tensor.matmul(
                            s_ps[:, j, css[j] : 512],
                            lhsT=kTs[b][
                                h0 : h0 + D, jb * 128 : (jb + 1) * 128
                            ],
                            rhs=qTs[b][h0 : h0 + D, q0l + css[j] : q0l + 512],
                            start=True,
                            stop=True,
                        )
                    sflat = s_ps.rearrange("p a f -> p (a f)")
                    if diag:
                        pp = (jbs[0] - 4 * i) // 2
                        e8s[p] = (e8d, 2 * pp)
                        nc.scalar.activation(
                            edflat[:, 1024 * pp + cs : 1024 * pp + 1024],
                            sflat[:, cs:1024],
                            Exp,
                            scale=0.125,
                        )
                        if pp == 1:
                            mflat = masks_sb.rearrange("p r f -> p (r f)")
                            nc.vector.tensor_mul(edflat, edflat, mflat)
                    else:
                        e8 = work.tile(
                            [128, 2, 512], BF16, tag="esb", bufs=12, name="e8"
                        )
                        e8s[p] = (e8, 0)
                        eflat = e8.rearrange("p a f -> p (a f)")
                        nc.scalar.activation(
                            eflat, sflat, Exp, scale=0.125
                        )
                    if idx >= 1 and deferred:
                        if idx < npair - 1:
                            deferred.pop(0)()
                            if deferred:
                                deferred.pop(0)()
                        else:
                            while deferred:
                                deferred.pop(0)()
                    pump()
                return e8s

            def attv2_burst(b, hl, i, o2, e8s, s):
                """One qsub's att@v accumulation: a single contiguous
                open-close psum group over key blocks 0..4i+s."""
                for kb in range(4 * i + s + 1):
                    p, j = divmod(kb, 2)
                    tile_, j0 = e8s[p]
                    nc.tensor.matmul(
                        o2[:, s, 0:65],
                        lhsT=tile_[:, j0 + j, 128 * s : 128 * (s + 1)],
                        rhs=va_sb[:, b * JBB + kb, hl, 0 : D + 1],
                        start=(kb == 0),
                        stop=(kb == 4 * i + s),
                        skip_group_check=True,
                    )

            def post_hl(b, hl, i, o2, att2, r_sb):
                """Normalize+drain one head's attv2 psum into attoT2; on the
                second head, chase each qsub's copy with its DMA-transpose
                into attoT [ch, tok] (two queues so the four transposes run
                pairwise-parallel)."""
                nc.vector.reciprocal(r_sb[:, :], o2[:, 0:4, 64])
                for s in range(4):
                    nc.vector.tensor_scalar_mul(
                        att2[:, s, h0c(hl)], o2[:, s, 0:64], r_sb[:, s : s + 1]
                    )
                if hl == 1:
                    # one batched xbar transpose flips all four [tok, ch]
                    # blocks into attoT's [ch, tok] layout
                    nc.sync.dma_start_transpose(
                        attoTs[(b, i)], att2.rearrange("p a f -> p (a f)")
                    )

            def h0c(hl):
                return slice(hl * D, (hl + 1) * D)

            def outproj_tile(b, tt, k):
                """One [128 tok, 1024 C] partial output projection block."""
                t0 = b * T
                tb = t0 // 128 + tt
                o_sb = work.tile(
                    [128, 2, 512], BF16, tag="osb", bufs=3, name="o_sb"
                )
                for no2 in range(2):
                    p_ps = ps_proj.tile([128, 512], F32, tag="proj", name="p_ps")
                    nc.tensor.matmul(
                        p_ps,
                        lhsT=attoTs[(b, tt // 4)][:, tt % 4, :],
                        rhs=wo_sb[:, no2 * 512 : (no2 + 1) * 512],
                        start=True,
                        stop=True,
                    )
                    # gpsimd can't read PSUM; scalar jitter would stall the
                    # scores ring through the exp chain: drains live on DVE
                    nc.vector.tensor_copy(o_sb[:, no2, :], p_ps)
                nc.gpsimd.dma_start(
                    out[tb * 128 : (tb + 1) * 128, :],
                    o_sb.rearrange("p a f -> p (a f)"),
                )

            # ================= schedule =================
            # two filler queues: outproj units (dependency-lagged, preferred)
            # and proj units (dependency-free but deadline-bound). Proj units
            # are only pumped when their deadline is near, reserving them as
            # dense filler for the outproj-poor attention groups they unlock.
            filler = []
            state = {"proj_done": 0, "opk": 0, "cur": 0}
            units = proj_units()

            def run_proj_unit():
                next(units)()
                state["proj_done"] += 1

            def pump():
                if filler:
                    filler.pop(0)()
                    return
                # proj unit n serves attn group n//3 (3 units per nt)
                if state["proj_done"] < 3 * B * NT and (
                    state["proj_done"] // 3 <= state["cur"] + 2
                ):
                    run_proj_unit()

            def force_proj(b, i):
                need = 3 * (i + 1) + (3 * NT if b == 1 else 0)
                while state["proj_done"] < need:
                    run_proj_unit()

            # startup: first projection tile of batch 0, no filler
            _s = nc.enter_named_scope("phaseA0", True)
            xload(0, 0)
            for _ in range(3):
                run_proj_unit()
            nc.scalar.dma_start(masks_sb, masks.rearrange("r p f -> p r f"))
            nc.scalar.dma_start(wo_sb[:, 0:512], wo[:, 0:512])
            nc.scalar.dma_start(wo_sb[:, 512:C], wo[:, 512:C])
            nc.leave_named_scope("phaseA0", _s[0], True)

            deferred = []
            staging = []
            for b in range(B):
                _s = nc.enter_named_scope(f"attn{b}", True)
                for i in range(QT):
                    state["cur"] = b * QT + i
                    force_proj(b, i)
                    if b == 0 and i == 0:
                        pump()
                    att2 = work.tile(
                        [128, 4, 128], BF16, tag="att2", bufs=2, name="att2"
                    )
                    for hl in range(HL):
                        filler.extend(staging)
                        staging.clear()
                        r_sb = work.tile(
                            [128, 4], F32, tag="rsb", bufs=4, name="r_sb"
                        )
                        o2 = ps_o.tile([128, 4, 128], F32, tag="ops", name="o2")
                        e8s = attn_pairs(b, hl, i, pump, deferred)
                        # defer this group's att@v + normalization into the
                        # next group's pair loop (past its exp latency)
                        deferred = [
                            (lambda b=b, hl=hl, i=i, o2=o2, e8s=e8s, s=s:
                                attv2_burst(b, hl, i, o2, e8s, s))
                            for s in range(4)
                        ]
                        deferred.append(
                            lambda b=b, hl=hl, i=i, o2=o2, att2=att2, r_sb=r_sb:
                                post_hl(b, hl, i, o2, att2, r_sb)
                        )
                        if hl == 1:
                            def fin(b=b, i=i):
                                # stage rather than release: outproj units
                                # become poppable one group later, past the
                                # copy->transpose chain of their attoT data
                                for tt in range(i * 4, (i + 1) * 4):
                                    k = state["opk"]
                                    state["opk"] += 1
                                    staging.append(
                                        lambda b=b, tt=tt, k=k:
                                            outproj_tile(b, tt, k)
                                    )
                            deferred.append(fin)
                nc.leave_named_scope(f"attn{b}", _s[0], True)

            _s = nc.enter_named_scope("tail", True)
            # backlog first: it covers the last group's exp/copy latency
            while state["proj_done"] < 3 * B * NT:
                run_proj_unit()
            while filler:
                filler.pop(0)()
            while deferred:
                deferred.pop(0)()
            filler.extend(staging)
            staging.clear()
            while filler:
                filler.pop(0)()
            nc.leave_named_scope("tail", _s[0], True)

    _split_waits(nc)
    return nc


def make_in_maps(x, Wq, bq, Wk, bk, Wv, bv, Wo, bo):
    xT = np.ascontiguousarray(x.reshape(TOK, C).T).astype(NPBF16)
    # masks[r, a, c] = 1 if c >= 128r + a  (causal within diagonal blocks)
    a = np.arange(128)[:, None]
    c = np.arange(512)[None, :]
    masks = np.stack(
        [(c >= 128 * rr + a).astype(NPBF16) for rr in range(4)]
    )
    in_maps = []
    for core in range(NCORES):
        sl = slice(core * HC, (core + 1) * HC)
        in_maps.append(
            {
                "xT": xT,
                "wq": np.ascontiguousarray(Wq[sl, :].T).astype(NPBF16),
                "wk": np.ascontiguousarray(Wk[sl, :].T).astype(NPBF16),
                "wv": np.ascontiguousarray(Wv[sl, :].T).astype(NPBF16),
                "wo": np.ascontiguousarray(Wo[:, sl].T).astype(NPBF16),
                "bq": np.ascontiguousarray(bq[sl]).reshape(HC, 1),
                "bk": np.ascontiguousarray(bk[sl]).reshape(HC, 1),
                "bv": np.ascontiguousarray(bv[sl]).reshape(1, HC).astype(NPBF16),
                "ones1": np.ones((1, 128), NPBF16),
                "masks": masks,
            }
        )
    return in_maps


_NC_CACHE = None


def kernel(x, Wq, bq, Wk, bk, Wv, bv, Wo, bo):
    global _NC_CACHE
    x = np.asarray(x, np.float32)
    in_maps = make_in_maps(
        x,
        np.asarray(Wq, np.float32),
        np.asarray(bq, np.float32),
        np.asarray(Wk, np.float32),
        np.asarray(bk, np.float32),
        np.asarray(Wv, np.float32),
        np.asarray(bv, np.float32),
        np.asarray(Wo, np.float32),
        np.asarray(bo, np.float32),
    )
    if _NC_CACHE is None:
        _NC_CACHE = build()
    trace = bool(int(os.environ.get("KERNEL_TRACE", "0")))
    res = run_bass_kernel_spmd(
        _NC_CACHE, in_maps, core_ids=list(range(NCORES)), trace=trace
    )
    if trace:
        kernel.last_results = res
    total = np.zeros((TOK, C), np.float32)
    for core in range(NCORES):
        total += res.results[core]["out"].astype(np.float32)
    total += np.asarray(bo, np.float32)[None, :]
    return total.reshape(B, T, C)


# revision 34
# speedup vs baseline: 1.0377x; 1.0091x over previous
"""Multi-head causal attention (B=2, T=2048, C=1024, H=16) on 8 trn2 cores.

Sharding: tensor-parallel over heads. Each core computes 2 heads' QKV
projections + attention + a partial output projection; the host sums the
8 partial projections and adds the output bias.

v3 design notes (on top of the v2 p-state/filler scheme):
- att@v runs TRANSPOSED: out[q,ch] = e8.T @ va with the exp'd score block
  as the stationary operand. Output partitions become q-tokens, the full
  128x128 array is used (contraction = 128 key tokens), halving att@v PE
  cycles vs the [ch, q] orientation (out free = 65 channels only).
- The softmax row-sum falls out of the same matmuls via the ones column
  in va (out col 64), already in token-partition layout. Normalization is
  a DVE reciprocal [128,4] plus a per-head tensor_scalar_mul fused into
  the PSUM->SBUF drain. The v2 selector-matmul + Ln/Exp reciprocal
  pipeline (~20us Scalar + 8k PE cycles) is gone entirely.
- attoT2 [tok, ch] is flipped back to [ch, tok] for the output projection
  with SBUF->SBUF DMA transposes (xbar 16x128 tiles, SP queue) - no PE
  transposes, no extra PSUM, no compute-engine copies.
- QKV projection of batch 1 and the output projections are interleaved
  into the attention loops as filler so PE never idles (idle gaps cost
  ~3.4us of half-clock ramp each). QKV is emitted per (nt: x+q, k, v)
  unit for finer pump granularity; attention starts right after nt=0 of
  batch 0 so the Scalar exp stream spans nearly the whole kernel.
- Engine placement: Scalar = exp only (+ a few DMA issues); DVE = bias
  copies, masks, recips, half the outproj drains; Pool = scaled attoT2
  copies + other half of outproj drains; SP = bulk DMA + transposes.
"""

import contextlib
import os

import ml_dtypes
import numpy as np

import bass_rust
import concourse.bass as bass
import concourse.mybir as mybir
import concourse.tile as tile
from concourse.bass_utils import run_bass_kernel_spmd

F32 = mybir.dt.float32
BF16 = mybir.dt.bfloat16
NPBF16 = ml_dtypes.bfloat16

B, T, C, H = 2, 2048, 1024, 16
D = C // H          # 64
NCORES = 8
HL = H // NCORES    # heads per core = 2
TOK = B * T         # 4096
HC = HL * D         # local head channels = 128

NT = T // 512       # 4 token column tiles (512) per batch
KT = C // 128       # 8 contraction tiles for projections
QT = T // 512       # 4 q tiles per batch
JBB = T // 128      # 16 j (key) blocks per batch

_MAXW = 1


def _patched_drain_and_barrier(self, tick_clock, wait_clock):
    """Stock tile tail drain carries one sem-wait per outstanding proc on a
    single TPB_CTRL drain; this walrus build allows only one sync-wait per
    ctrl instruction. Split the waits across no-op carriers."""
    nc = self.nc
    carrier = nc.sync.nop()
    wait_clock.add_sem_waits(
        carrier.ins, bass_rust.ScopedClock({None: tick_clock.global_clock})
    )
    si = carrier.ins.sync_info
    waits = list(si.on_wait) if si and si.on_wait else []
    if len(waits) > _MAXW:
        carrier.ins.sync_info = mybir.SyncInfo(
            on_wait=waits[:_MAXW], on_update=list(si.on_update or [])
        )
        for i in range(_MAXW, len(waits), _MAXW):
            nop = nc.sync.nop()
            nop.ins.sync_info = mybir.SyncInfo(
                on_wait=waits[i : i + _MAXW], on_update=[]
            )
    nc.sync.drain()

    nc.all_engine_barrier()
    popped = nc._tile_sem_poison_stack.pop()
    assert popped is self._sem_poison
    assert self.sems is not None
    nc.clear_and_free_semaphores(list(self.sems.allocated().values()))
    nc.all_engine_barrier()


tile.TileContext._drain_and_barrier = _patched_drain_and_barrier


def _split_waits(nc, maxw=_MAXW):
    """This walrus build accepts at most one sync-wait per instruction.
    Hoist excess waits onto no-op carriers inserted just before the
    instruction on the same engine."""
    for f in nc.m.functions:
        for bb in f.blocks:
            insts = bb.instructions
            if not any(
                i.sync_info and i.sync_info.on_wait and len(i.sync_info.on_wait) > maxw
                for i in insts
            ):
                continue
            new = []
            for inst in insts:
                si = inst.sync_info
                waits = list(si.on_wait) if si and si.on_wait else []
                if len(waits) > maxw:
                    keep = waits[-maxw:]
                    extra = waits[:-maxw]
                    for j in range(0, len(extra), maxw):
                        nop = mybir.InstNoOp(name=nc.get_next_instruction_name())
                        nop.engine = inst.engine
                        nop.sync_info = mybir.SyncInfo(
                            on_wait=extra[j : j + maxw], on_update=[]
                        )
                        nc.register_instruction(nop)
                        new.append(nop)
                    inst.sync_info = mybir.SyncInfo(
                        on_wait=keep, on_update=list(si.on_update or [])
                    )
                new.append(inst)
            bb.instructions = new


def build():
    nc = bass.Bass()
    xT = nc.declare_dram_parameter("xT", [C, TOK], BF16, isOutput=False)
    wq = nc.declare_dram_parameter("wq", [C, HC], BF16, isOutput=False)
    wk = nc.declare_dram_parameter("wk", [C, HC], BF16, isOutput=False)
    wv = nc.declare_dram_parameter("wv", [C, HC], BF16, isOutput=False)
    wo = nc.declare_dram_parameter("wo", [HC, C], BF16, isOutput=False)
    bq = nc.declare_dram_parameter("bq", [HC, 1], F32, isOutput=False)
    bk = nc.declare_dram_parameter("bk", [HC, 1], F32, isOutput=False)
    bv = nc.declare_dram_parameter("bv", [1, HC], BF16, isOutput=False)
    ones1 = nc.declare_dram_parameter("ones1", [1, 128], BF16, isOutput=False)
    masks = nc.declare_dram_parameter("masks", [4, 128, 512], BF16, isOutput=False)
    out = nc.declare_dram_parameter("out", [TOK, C], BF16, isOutput=True)

    Exp = mybir.ActivationFunctionType.Exp

    with contextlib.ExitStack() as _st:
        _st.enter_context(
            nc.allow_low_precision(reason="bf16 matmuls with fp32 accumulation")
        )
        tc = _st.enter_context(tile.TileContext(nc))
        with (
            tc.tile_pool(name="consts", bufs=1) as consts,
            tc.tile_pool(name="persist", bufs=1) as persist,
            tc.tile_pool(name="work", bufs=2) as work,
            tc.tile_pool(name="ps_proj", bufs=2, space="PSUM") as ps_proj,
            tc.tile_pool(name="ps_s", bufs=2, space="PSUM") as ps_s,
            tc.tile_pool(name="ps_o", bufs=2, space="PSUM") as ps_o,
        ):
            # ---- constants into SBUF ----
            wq_sb = consts.tile([128, KT, 128], BF16, name="wq_sb")
            wk_sb = consts.tile([128, KT, 128], BF16, name="wk_sb")
            wv_sb = consts.tile([128, KT, 128], BF16, name="wv_sb")
            # first chunks of each weight lead so the first projection
            # matmuls start early; small consts + masks on the scalar queue
            # (idle until the first exp), wo (needed last) at the back
            wtriples = [
                (wq_sb, wq.rearrange("(a p) m -> p a m", p=128)),
                (wk_sb, wk.rearrange("(a p) m -> p a m", p=128)),
                (wv_sb, wv.rearrange("(a p) m -> p a m", p=128)),
            ]
            for w_sb, wr in wtriples:
                nc.sync.dma_start(w_sb[:, 0:2], wr[:, 0:2])
            for w_sb, wr in wtriples:
                nc.sync.dma_start(w_sb[:, 2:5], wr[:, 2:5])
                nc.sync.dma_start(w_sb[:, 5:KT], wr[:, 5:KT])
            wo_sb = consts.tile([128, C], BF16, name="wo_sb")
            bq_sb = consts.tile([HC, 1], F32, name="bq_sb")
            bk_sb = consts.tile([HC, 1], F32, name="bk_sb")
            for b_sb, b_dr in ((bq_sb, bq), (bk_sb, bk)):
                nc.scalar.dma_start(b_sb, b_dr[:])
            bv_sb = consts.tile([1, HC], BF16, name="bv_sb")
            nc.scalar.dma_start(bv_sb, bv[:])
            ones1_sb = consts.tile([1, 128], BF16, name="ones1_sb")
            nc.scalar.dma_start(ones1_sb, ones1[:])
            masks_sb = consts.tile([128, 4, 512], BF16, name="masks_sb")

            # ---- persistent activations ----
            # per-batch / per-qtile tiles: one big tile would funnel all
            # readers+writers through the dep tracker's conservative
            # whole-tile fallback (WAR on late batch-0 outproj reads was
            # stalling batch-1 transposes ~19us, head-of-line blocking the
            # issuing engine queue behind them)
            qTs = [persist.tile([HC, T], BF16, name=f"qT{b}") for b in range(B)]
            kTs = [persist.tile([HC, T], BF16, name=f"kT{b}") for b in range(B)]
            attoTs = {
                (b, i): persist.tile([HC, 4, 128], BF16, name=f"attoT{b}_{i}")
                for b in range(B)
                for i in range(QT)
            }
            # va: per local head, [key-token partitions, 32 global blocks,
            # 64 v channels + ones column], pitch 80
            va_sb = persist.tile([128, 2 * JBB, HL, 80], BF16, name="va_sb")
            ones_scr = consts.tile([128, 2 * JBB, HL], BF16, name="ones_scr")
            nc.vector.memset(ones_scr, 1.0)
            nc.vector.tensor_copy(va_sb[:, :, :, D], ones_scr)

            # ================= emission units =================

            xTr = xT.rearrange("(a p) m -> p a m", p=128)
            xpend = {}

            def xload(b, nt):
                """Prefetch one 512-token x tile (issued one proj unit ahead
                so the q matmuls never wait on HBM)."""
                c0 = b * T + nt * 512
                xrow = work.tile(
                    [128, KT, 512], BF16, tag="xcol", bufs=3, name="xrow"
                )
                nc.gpsimd.dma_start(xrow[:, 0:2], xTr[:, 0:2, c0 : c0 + 512])
                nc.gpsimd.dma_start(xrow[:, 2:KT], xTr[:, 2:KT, c0 : c0 + 512])
                xpend[(b, nt)] = xrow

            def unit_xq(b, nt):
                """Q projection for one 512-token tile (x already in SBUF)."""
                xrow = xpend.pop((b, nt))
                nxt = (b, nt + 1) if nt + 1 < NT else (b + 1, 0)
                if nxt[0] < B:
                    xload(*nxt)
                q_ps = ps_proj.tile([128, 512], F32, tag="proj", name="q_ps")
                for kt in range(KT):
                    nc.tensor.matmul(
                        q_ps, lhsT=wq_sb[:, kt, :], rhs=xrow[:, kt, :],
                        start=(kt == 0), stop=(kt == KT - 1),
                    )
                nc.vector.tensor_scalar_add(
                    qTs[b][:, nt * 512 : nt * 512 + 512], q_ps, bq_sb
                )
                return xrow

            def unit_k(b, nt, xrow):
                k_ps = ps_proj.tile([128, 512], F32, tag="proj", name="k_ps")
                for kt in range(KT):
                    nc.tensor.matmul(
                        k_ps, lhsT=wk_sb[:, kt, :], rhs=xrow[:, kt, :],
                        start=(kt == 0), stop=(kt == KT - 1),
                    )
                nc.vector.tensor_scalar_add(
                    kTs[b][:, nt * 512 : nt * 512 + 512], k_ps, bk_sb
                )

            def unit_v(b, nt, xrow):
                """v in [token, channel] layout: x block as stationary."""
                for blk in range(4):
                    gblk = b * JBB + nt * 4 + blk
                    va_ps = ps_proj.tile([128, 128], F32, tag="proj", name="va_ps")
                    nc.tensor.matmul(
                        va_ps, lhsT=ones1_sb, rhs=bv_sb, start=True, stop=False
                    )
                    for kt in range(KT):
                        nc.tensor.matmul(
                            va_ps,
                            lhsT=xrow[:, kt, blk * 128 : (blk + 1) * 128],
                            rhs=wv_sb[:, kt, :],
                            start=False,
                            stop=(kt == KT - 1),
                        )
                    nc.vector.tensor_copy(
                        va_sb[:, gblk, :, 0:D],
                        va_ps.rearrange("p (h d) -> p h d", h=HL),
                    )

            def proj_units():
                for b in range(B):
                    for nt in range(NT):
                        holder = {}

                        def uxq(b=b, nt=nt, holder=holder):
                            holder["x"] = unit_xq(b, nt)

                        yield uxq
                        yield lambda b=b, nt=nt, holder=holder: unit_k(b, nt, holder["x"])
                        yield lambda b=b, nt=nt, holder=holder: unit_v(b, nt, holder["x"])

            def attn_pairs(b, hl, i, pump, deferred):
                """Scores+exp for one (batch, local head, 512-query tile).
                Diagonal pairs first. The previous group's deferred att@v
                bursts are emitted after pair 1's scores so the exps they
                depend on are comfortably drained. Returns e8 tiles by pair.

                PSUM accumulation groups may not interleave their open spans
                within one bank (verified on hw: interleaved start/stop at
                different offsets of one bank corrupts all groups that don't
                close on the bank's final matmul). att@v therefore runs
                s-major as contiguous bursts - one open group at a time -
                pipelined one group behind the scores/exp stream."""
                t0 = b * T
                h0 = hl * D
                q0 = t0 + i * 512
                order = [2 * i, 2 * i + 1] + list(range(0, 2 * i))
                npair = len(order)
                # both diagonal pairs share one e8 tile so a single DVE mul
                # masks all four triangle blocks (the [1024:1280] gap region
                # is exp'd garbage that no burst ever reads)
                e8d = work.tile([128, 4, 512], BF16, tag="esd", bufs=4, name="e8d")
                edflat = e8d.rearrange("p a f -> p (a f)")
                e8s = {}
                for idx, p in enumerate(order):
                    jbs = (2 * p, 2 * p + 1)
                    css = [max(0, 128 * (jb - 4 * i)) for jb in jbs]
                    cs = css[0]
                    diag = jbs[0] >= 4 * i
                    s_ps = ps_s.tile([128, 2, 512], F32, tag="sps", name="s_ps")
                    q0l = i * 512
                    for j, jb in enumerate(jbs):
                        nc.tensor.matmul(
                            s_ps[:, j, css[j] : 512],
                            lhsT=kTs[b][
                                h0 : h0 + D, jb * 128 : (jb + 1) * 128
                            ],
                            rhs=qTs[b][h0 : h0 + D, q0l + css[j] : q0l + 512],
                            start=True,
                            stop=True,
                        )
                    sflat = s_ps.rearrange("p a f -> p (a f)")
                    if diag:
                        pp = (jbs[0] - 4 * i) // 2
                        e8s[p] = (e8d, 2 * pp)
                        nc.scalar.activation(
                            edflat[:, 1024 * pp + cs : 1024 * pp + 1024],
                            sflat[:, cs:1024],
                            Exp,
                            scale=0.125,
                        )
                        if pp == 1:
                            mflat = masks_sb.rearrange("p r f -> p (r f)")
                            nc.vector.tensor_mul(edflat, edflat, mflat)
                    else:
                        e8 = work.tile(
                            [128, 2, 512], BF16, tag="esb", bufs=12, name="e8"
                        )
                        e8s[p] = (e8, 0)
                        eflat = e8.rearrange("p a f -> p (a f)")
                        nc.scalar.activation(
                            eflat, sflat, Exp, scale=0.125
                        )
                    if idx >= 1 and deferred:
                        if idx < npair - 1:
                            deferred.pop(0)()
                            if deferred:
                                deferred.pop(0)()
                        else:
                            while deferred:
                                deferred.pop(0)()
                    pump()
                return e8s

            def attv2_burst(b, hl, i, o2, e8s, s):
                """One qsub's att@v accumulation: a single contiguous
                open-close psum group over key blocks 0..4i+s."""
                for kb in range(4 * i + s + 1):
                    p, j = divmod(kb, 2)
                    tile_, j0 = e8s[p]
                    nc.tensor.matmul(
                        o2[:, s, 0:65],
                        lhsT=tile_[:, j0 + j, 128 * s : 128 * (s + 1)],
                        rhs=va_sb[:, b * JBB + kb, hl, 0 : D + 1],
                        start=(kb == 0),
                        stop=(kb == 4 * i + s),
                        skip_group_check=True,
                    )

            def post_hl(b, hl, i, o2, att2, r_sb):
                """Normalize+drain one head's attv2 psum into attoT2; on the
                second head, chase each qsub's copy with its DMA-transpose
                into attoT [ch, tok] (two queues so the four transposes run
                pairwise-parallel)."""
                nc.vector.reciprocal(r_sb[:, :], o2[:, 0:4, 64])
                for s in range(4):
                    nc.vector.tensor_scalar_mul(
                        att2[:, s, h0c(hl)], o2[:, s, 0:64], r_sb[:, s : s + 1]
                    )
                if hl == 1:
                    # one batched xbar transpose flips all four [tok, ch]
                    # blocks into attoT's [ch, tok] layout
                    nc.sync.dma_start_transpose(
                        attoTs[(b, i)], att2.rearrange("p a f -> p (a f)")
                    )

            def h0c(hl):
                return slice(hl * D, (hl + 1) * D)

            def outproj_tile(b, tt, k):
                """One [128 tok, 1024 C] partial output projection block."""
                t0 = b * T
                tb = t0 // 128 + tt
                o_sb = work.tile(
                    [128, 2, 512], BF16, tag="osb", bufs=3, name="o_sb"
                )
                for no2 in range(2):
                    p_ps = ps_proj.tile([128, 512], F32, tag="proj", name="p_ps")
                    nc.tensor.matmul(
                        p_ps,
                        lhsT=attoTs[(b, tt // 4)][:, tt % 4, :],
                        rhs=wo_sb[:, no2 * 512 : (no2 + 1) * 512],
                        start=True,
                        stop=True,
                    )
                    # gpsimd can't read PSUM; scalar jitter would stall the
                    # scores ring through the exp chain: drains live on DVE
                    nc.vector.tensor_copy(o_sb[:, no2, :], p_ps)
                nc.gpsimd.dma_start(
                    out[tb * 128 : (tb + 1) * 128, :],
                    o_sb.rearrange("p a f -> p (a f)"),
                )

            # ================= schedule =================
            # two filler queues: outproj units (dependency-lagged, preferred)
            # and proj units (dependency-free but deadline-bound). Proj units
            # are only pumped when their deadline is near, reserving them as
            # dense filler for the outproj-poor attention groups they unlock.
            filler = []
            state = {"proj_done": 0, "opk": 0, "cur": 0}
            units = proj_units()

            def run_proj_unit():
                next(units)()
                state["proj_done"] += 1

            def pump():
                if filler:
                    filler.pop(0)()
                    return
                # proj unit n serves attn group n//3 (3 units per nt)
                if state["proj_done"] < 3 * B * NT and (
                    state["proj_done"] // 3 <= state["cur"] + 2
                ):
                    run_proj_unit()

            def force_proj(b, i):
                need = 3 * (i + 1) + (3 * NT if b == 1 else 0)
                while state["proj_done"] < need:
                    run_proj_unit()

            # startup: first projection tile of batch 0, no filler
            _s = nc.enter_named_scope("phaseA0", True)
            xload(0, 0)
            for _ in range(3):
                run_proj_unit()
            nc.scalar.dma_start(masks_sb, masks.rearrange("r p f -> p r f"))
            nc.scalar.dma_start(wo_sb[:, 0:512], wo[:, 0:512])
            nc.scalar.dma_start(wo_sb[:, 512:C], wo[:, 512:C])
            nc.leave_named_scope("phaseA0", _s[0], True)

            deferred = []
            staging = []
            for b in range(B):
                _s = nc.enter_named_scope(f"attn{b}", True)
                for i in range(QT):
                    state["cur"] = b * QT + i
                    force_proj(b, i)
                    att2 = work.tile(
                        [128, 4, 128], BF16, tag="att2", bufs=2, name="att2"
                    )
                    for hl in range(HL):
                        filler.extend(staging)
                        staging.clear()
                        r_sb = work.tile(
                            [128, 4], F32, tag="rsb", bufs=4, name="r_sb"
                        )
                        o2 = ps_o.tile([128, 4, 128], F32, tag="ops", name="o2")
                        e8s = attn_pairs(b, hl, i, pump, deferred)
                        # defer this group's att@v + normalization into the
                        # next group's pair loop (past its exp latency)
                        deferred = [
                            (lambda b=b, hl=hl, i=i, o2=o2, e8s=e8s, s=s:
                                attv2_burst(b, hl, i, o2, e8s, s))
                            for s in range(4)
                        ]
                        deferred.append(
                            lambda b=b, hl=hl, i=i, o2=o2, att2=att2, r_sb=r_sb:
                                post_hl(b, hl, i, o2, att2, r_sb)
                        )
                        if hl == 1:
                            def fin(b=b, i=i):
                                # stage rather than release: outproj units
                                # become poppable one group later, past the
                                # copy->transpose chain of their attoT data
                                for tt in range(i * 4, (i + 1) * 4):
                                    k = state["opk"]
                                    state["opk"] += 1
                                    staging.append(
                                        lambda b=b, tt=tt, k=k:
                                            outproj_tile(b, tt, k)
                                    )
                            deferred.append(fin)
                nc.leave_named_scope(f"attn{b}", _s[0], True)

            _s = nc.enter_named_scope("tail", True)
            # backlog first: it covers the last group's exp/copy latency
            while state["proj_done"] < 3 * B * NT:
                run_proj_unit()
            while filler:
                filler.pop(0)()
            while deferred:
                deferred.pop(0)()
            filler.extend(staging)
            staging.clear()
            while filler:
                filler.pop(0)()
            nc.leave_named_scope("tail", _s[0], True)

    _split_waits(nc)
    return nc


def make_in_maps(x, Wq, bq, Wk, bk, Wv, bv, Wo, bo):
    xT = np.ascontiguousarray(x.reshape(TOK, C).T).astype(NPBF16)
    # masks[r, a, c] = 1 if c >= 128r + a  (causal within diagonal blocks)
    a = np.arange(128)[:, None]
    c = np.arange(512)[None, :]
    masks = np.stack(
        [(c >= 128 * rr + a).astype(NPBF16) for rr in range(4)]
    )
    in_maps = []
    for core in range(NCORES):
        sl = slice(core * HC, (core + 1) * HC)
        in_maps.append(
            {
                "xT": xT,
                "wq": np.ascontiguousarray(Wq[sl, :].T).astype(NPBF16),
                "wk": np.ascontiguousarray(Wk[sl, :].T).astype(NPBF16),
                "wv": np.ascontiguousarray(Wv[sl, :].T).astype(NPBF16),
                "wo": np.ascontiguousarray(Wo[:, sl].T).astype(NPBF16),
                "bq": np.ascontiguousarray(bq[sl]).reshape(HC, 1),
                "bk": np.ascontiguousarray(bk[sl]).reshape(HC, 1),
                "bv": np.ascontiguousarray(bv[sl]).reshape(1, HC).astype(NPBF16),
                "ones1": np.ones((1, 128), NPBF16),
                "masks": masks,
            }
        )
    return in_maps


_NC_CACHE = None


def kernel(x, Wq, bq, Wk, bk, Wv, bv, Wo, bo):
    global _NC_CACHE
    x = np.asarray(x, np.float32)
    in_maps = make_in_maps(
        x,
        np.asarray(Wq, np.float32),
        np.asarray(bq, np.float32),
        np.asarray(Wk, np.float32),
        np.asarray(bk, np.float32),
        np.asarray(Wv, np.float32),
        np.asarray(bv, np.float32),
        np.asarray(Wo, np.float32),
        np.asarray(bo, np.float32),
    )
    if _NC_CACHE is None:
        _NC_CACHE = build()
    trace = bool(int(os.environ.get("KERNEL_TRACE", "0")))
    res = run_bass_kernel_spmd(
        _NC_CACHE, in_maps, core_ids=list(range(NCORES)), trace=trace
    )
    if trace:
        kernel.last_results = res
    total = np.zeros((TOK, C), np.float32)
    for core in range(NCORES):
        total += res.results[core]["out"].astype(np.float32)
    total += np.asarray(bo, np.float32)[None, :]
    return total.reshape(B, T, C)
